# revision 58
# baseline (speedup 1.0000x reference)
"""Trainium2 Bass kernel for a 2-layer GCN (EnhancedGNN) — v2.

Computation (eval mode):
    src,dst,norm = gcn_norm(edge_index)            # sym deg^-1/2 with self loops
    h  = relu(gcn_layer(x, W1, b1))
    h  = gcn_layer(h, W2, b2)
    out = sigmoid(h @ Wl + bl)

Identity: the per-edge norm dinv[src]*dinv[dst] factors into per-node row
scales, so  layer(X) = dinv * segsum(hs[src] -> dst) + b  with
hs = dinv * (X @ W); the added self loop contributes hs[own].

v2 design:
 - Nodes dealt to 8 cores by sorted in-degree; per-node source-side chosen by
   a global bipartition balancing each dst's in-edges across the two gather
   banks; nodes dealt to cores sorted by (side, need-class) so per-slot need
   profiles align across cores (shared static chunk layout, low padding).
 - Self loops handled by one identity matmul per group from the SBUF copy of
   hs (no gather slots).
 - Segment-sum on TensorE in transposed orientation psT[feat, node]: per
   128-slot chunk, matmul(lhsT=messages, rhs=selection).  Transposed output
   feeds the next layer's X@W and the final Wl reduction without transposes.
 - Messages fetched by batched dma_gather windows (direct mode; the
   prepare_only path's Q7 desc-gen measured ~7.8ns/idx vs ~1.3ns/idx here,
   so direct mode wins despite holding Pool during the transfer).
 - Each AllGather split in two sub-collectives (separate shard tensors) so
   message passing overlaps the collective.

v3 (632us, from 649us): GW=8 windows (finer queue pipelining), chunk
matmuls stream only their dst-column slice (first mm per group stays
full-width start=True to clear stale PSUM), const loads split across the
two HWDGE rings.  Measured wall: the gather stream runs ~155GB/s
(latency-bound 256B random HBM reads, 4 SWDGE queues max); tried and
rejected: single_packet=True (hangs), prepare_only prefetch (+15us),
warm-up barrier collective (+9us), 4-piece AGs (~10us fixed/op on the
serial CC stream), fully-replicated phase B with local bank writes (no
AG1, +2.8% desc load: phase-B slab pipeline couldn't beat the AG wire,
650us best — see kernel_repb_backup.py.bak).

v4 (~620us typical, best draw 602us; run variance +-2%): node dealing
key changed from (-max(n0,n1), -min) to lexicographic (-n1, -n0) — slot
positions then hold near-identical (n0,n1) pairs across cores, cutting
the cross-core max-need alignment overhead from ~10% to ~0.4% and total
gather slots 114432 -> 107008 (-6.5% stream bytes).  Also rejected on
measurement: staggered window sizes to de-phase the 4 queue-refill gaps
(fewer stream holes but lower burst rate, net +12us), and layer-2
piece-major banks with 4 AG pieces + second idx set (+74us — every
AG-splitting variant loses ~10us fixed/op on the serial CC stream).

v5 (~610us median, best draw 599us): phase-B bank writes batched into
one DMA per side (2 sync-ring issue slots instead of 49).  Startup is
now ~99us and hard: ~21us NEFF start, ~32us first-collective rendezvous
barrier, ~11us CC-stream latency, ~34us AG1 bank-0 wire — the AG trigger
already fires before the barrier ends, so nothing in-kernel moves it.
Also rejected: GBUFS=24 and shrinking the last pass's final windows
(both within run noise, slightly negative); idx=-1 trailing padding
(+60us, slow ucode path despite correct results); a single hot fake row
for padding (+230us, same-address fetches from 8 cores serialize an HBM
channel — keep the round-robin spread).  Run-to-run variance on this
setup is +-3-4%; only deltas >15us are attributable.

v6 findings (median ~604us, best draw 594.1us): GW sweep completed — {4:656, 6:635,
8:~607, 12:609, 16:607-650} — the SWDGE ring is PROGRAM-granular (one
outstanding window per queue regardless of descriptor count), so
smaller windows cannot double-buffer the ~3us ring-refill gap; GW=8 is
the optimum.  The ~19us GpSimd DRAIN near the tail runs on a spare Q7
core concurrently with the last windows (harmless).

v7 (best draw 591.7us): asymmetric banks GA=22/GB=27 — shrinks the two
critical-path AG wires (AG1-b0 gating stream start, AG2-b0 gating the
layer transition) by ~12% each for +256 slots; host-side estimate -6us,
first sample 591.7 (new best).
"""

import os
import sys

sys.path.insert(0, "/opt/trn_rl_repo")

import numpy as np

import concourse.bacc as bacc
import concourse.bass as bass
import concourse.tile as tile
from concourse import mybir
from concourse.bass_utils import run_bass_kernel_spmd

# ---------------------------------------------------------------- constants
N_REAL = 50000
E_EDGES = 800000
D = 128
NC = 8
G = 49
SHARD = G * 128              # 6272 slots per core
SHARD_REAL = N_REAL // NC    # 6250
# Asymmetric bank split: bank A (pass 0) smaller, so the two
# critical-path AllGather wires (AG1 bank-0 gating stream start, AG2
# bank-0 gating the layer transition) each shrink ~12%% (-4us each),
# for +256 gather slots (+0.8us).  GB<=31 keeps NC*SB under int16.
GA = 22                      # groups in bank A per core
GB = G - GA                  # 27
SA = GA * 128                # 2816
SB = GB * 128                # 3456
BANK_ROWS = (NC * SA, NC * SB)   # 22528, 27648 (< 32768 for int16)

GW = int(os.environ.get("GNN_GW", "8"))      # chunks per gather window
GBUFS = int(os.environ.get("GNN_GBUFS", "16"))  # gather buffers in flight
NQ = 4                       # SWDGE queues (ucode max)
SP = bool(int(os.environ.get("GNN_SP", "0")))   # single_packet for dma_gather

# AllGather sub-pieces per bank (1 = one AG per bank; 2 = split each bank's
# AG in two so pieces land earlier / trigger sooner).  Bank rows are laid
# out piece-major: [piece][rank][slot-within-piece].
AGP = int(os.environ.get("GNN_AGP", "1"))
if AGP == 1:
    PB = ([0, SA], [0, SB])          # piece slot boundaries within each side
else:
    PB = ([0, 13 * 128, SA], [0, 12 * 128, SB])


def _rows_of(side_arr, owner_arr, slot_arr):
    """Bank row for (side, owner, slot) under the piece-major layout."""
    rows = np.empty(len(side_arr), dtype=np.int64)
    for s_val, pb, base_off in ((0, PB[0], 0), (1, PB[1], SA)):
        m = side_arr == s_val
        t = slot_arr[m] - base_off
        pi = np.searchsorted(pb, t, side="right") - 1
        lo = np.asarray(pb)[pi]
        sz = np.asarray(pb)[pi + 1] - lo
        rows[m] = NC * lo + owner_arr[m] * sz + (t - lo)
    return rows

F32 = mybir.dt.float32
BF16 = mybir.dt.bfloat16
I16 = mybir.dt.int16
A = mybir.ActivationFunctionType


# ===================================================================== host
def _bipartition(src, dst, n_refine=8):
    """Side (0/1) per node as a SOURCE, balancing each dst's in-edges."""
    out_order = np.argsort(src, kind="stable")
    o_dst = dst[out_order]
    o_src = src[out_order]
    o_starts = np.searchsorted(o_src, np.arange(N_REAL + 1), side="left")

    cap = [NC * (SA - 4), NC * (SB - 4)]
    cnt = [0, 0]
    imb = np.zeros(N_REAL, dtype=np.int64)
    side = np.zeros(N_REAL, dtype=np.int8)

    out_deg = np.diff(o_starts)
    order = np.argsort(-out_deg, kind="stable")
    for v in order:
        nbrs = o_dst[o_starts[v]:o_starts[v + 1]]
        if cnt[0] >= cap[0]:
            s = 1
        elif cnt[1] >= cap[1]:
            s = 0
        elif len(nbrs) == 0:
            s = 0 if cnt[0] * cap[1] <= cnt[1] * cap[0] else 1
        else:
            ib = imb[nbrs]
            dA = np.abs(ib + 1).sum() - np.abs(ib).sum()
            dB = np.abs(ib - 1).sum() - np.abs(ib).sum()
            if dA != dB:
                s = 0 if dA < dB else 1
            else:
                s = 0 if cnt[0] * cap[1] <= cnt[1] * cap[0] else 1
        side[v] = s
        cnt[s] += 1
        imb[nbrs] += 1 - 2 * s

    for _ in range(n_refine):
        flips = 0
        for v in range(N_REAL):
            nbrs = o_dst[o_starts[v]:o_starts[v + 1]]
            if len(nbrs) == 0:
                continue
            s = side[v]
            delta = -2 * (1 - 2 * s)
            gain = (np.abs(imb[nbrs] + delta) - np.abs(imb[nbrs])).sum()
            if gain < 0 and cnt[1 - s] < cap[1 - s]:
                side[v] = 1 - s
                cnt[s] -= 1
                cnt[1 - s] += 1
                imb[nbrs] += delta
                flips += 1
        if flips < 20:
            break
    return side


def _dp_chunks(prof):
    """Min-#chunks cover of a group's need profile.  Chunk = 128 gathered
    slots holding up to 128//d nodes at d slots each."""
    n = len(prof)
    last = n
    while last > 0 and prof[last - 1] == 0:
        last -= 1
    if last == 0:
        return []
    INF = 1 << 30
    f = [INF] * (last + 1)
    choice = [None] * (last + 1)
    f[last] = 0
    for i in range(last - 1, -1, -1):
        mx = 0
        for j in range(i + 1, min(i + 129, last + 1)):
            if prof[j - 1] > mx:
                mx = prof[j - 1]
            d = mx if mx > 0 else 1
            if (j - i) > 128 // d:
                break
            if 1 + f[j] < f[i]:
                f[i] = 1 + f[j]
                choice[i] = (j, d)
    chunks = []
    i = 0
    while i < last:
        j, d = choice[i]
        chunks.append((d, i, j - i))
        i = j
    return chunks


def _host_prep(x, edge_index):
    src = np.asarray(edge_index[0], dtype=np.int64)
    dst = np.asarray(edge_index[1], dtype=np.int64)

    deg_in = np.bincount(dst, minlength=N_REAL)
    deg_n = deg_in + 1                                  # + self loop
    dinv = (1.0 / np.sqrt(deg_n)).astype(np.float32)

    side = _bipartition(src, dst)

    # per-dst needs by source side
    src_side = side[src].astype(np.int64)
    n0 = np.zeros(N_REAL, dtype=np.int64)
    np.add.at(n0, dst[src_side == 0], 1)
    n1 = deg_in - n0

    # ---- deal nodes to cores by (side, class) so core profiles align
    owner = np.empty(N_REAL, dtype=np.int64)
    slot = np.empty(N_REAL, dtype=np.int64)
    node_of_slot = np.full((NC, SHARD), -1, dtype=np.int64)
    for s_val, base in ((0, 0), (1, SA)):
        nodes = np.nonzero(side == s_val)[0]
        # lexicographic by the ordered pair (primary n1, secondary n0):
        # positions then hold near-identical (n0, n1) across cores, so the
        # cross-core max-need profile hugs the mean (alignment overhead
        # ~0.4% vs ~10% for the old (max,min) key) -> ~6.5% fewer gather
        # slots.
        key = np.lexsort((-n0[nodes], -n1[nodes]))
        nodes = nodes[key]
        c = np.arange(len(nodes)) % NC
        pos = np.arange(len(nodes)) // NC
        owner[nodes] = c
        slot[nodes] = base + pos
        node_of_slot[c, base + pos] = nodes

    # gather row of node v within its bank (per-layer layouts)
    bank_of = side.astype(np.int64)
    row_in_bank = _rows_of(side.astype(np.int64), owner, slot)

    # ---- needs per (core, slot, pass), cross-core max, DP chunk layout
    needs = np.zeros((NC, SHARD, 2), dtype=np.int64)
    np.add.at(needs, (owner[dst], slot[dst], src_side), 1)
    max_need = needs.max(axis=0)

    # chunks may straddle one group boundary (two matmuls share the gather)
    sched = []          # (p, d, abs_base, take, off)
    off = 0
    d_vals = set()
    has_p = np.zeros((2, G), dtype=bool)
    for p in range(2):
        for (d, base, take) in _dp_chunks(list(max_need[:, p])):
            sched.append((p, d, base, take, off))
            off += 128
            d_vals.add(d)
            for g in range(base // 128, (base + take - 1) // 128 + 1):
                has_p[p, g] = True
    tot_slots = off
    d_set = sorted(d_vals)


    # ---- fake (all-zero) rows per bank, global across cores
    fake_rows = []
    for p, base, size in ((0, 0, SA), (1, SA, SB)):
        rows = []
        for c in range(NC):
            fs = np.nonzero(node_of_slot[c, base:base + size] < 0)[0]
            if len(fs):
                pp = np.full(len(fs), p, dtype=np.int64)
                cc = np.full(len(fs), c, dtype=np.int64)
                rows.extend(_rows_of(pp, cc, base + fs))
        fake_rows.append(np.array(sorted(rows), dtype=np.int64))
        assert len(rows) > 0

    # ---- per-core CSR of in-edges sorted by (dst)
    e_order = np.argsort(dst, kind="stable")
    s_src = src[e_order]
    s_dst = dst[e_order]
    e_starts = np.searchsorted(s_dst, np.arange(N_REAL + 1), side="left")

    # ---- per-core gather indices
    idx_maps = []
    for c in range(NC):
        idx_flat = np.empty(tot_slots, dtype=np.int16)
        rr = 0
        for (p, d, base, take, o) in sched:
            fl = fake_rows[p]
            blk = fl[(rr + np.arange(128)) % len(fl)].astype(np.int64)
            rr += 128
            for t in range(take):
                v = node_of_slot[c, base + t]
                if v < 0:
                    continue
                e0, e1 = e_starts[v], e_starts[v + 1]
                srcs = s_src[e0:e1]
                rows = np.sort(row_in_bank[srcs[side[srcs] == p]])
                assert len(rows) <= d, (len(rows), d)
                blk[t * d:t * d + len(rows)] = rows
            idx_flat[o:o + 128] = blk.astype(np.int16)
        idx_maps.append(np.tile(
            idx_flat.reshape(-1, 16).T.copy(), (8, 1)))  # [128, tot/16]

    # ---- per-core dense inputs
    xT_maps, dinvb_maps, dinvc_maps = [], [], []
    x = np.asarray(x, dtype=np.float32)
    for c in range(NC):
        nodes = node_of_slot[c]
        mask = nodes >= 0
        xT = np.zeros((D, SHARD), dtype=np.float32)
        xT[:, mask] = x[nodes[mask]].T
        xT_maps.append(np.ascontiguousarray(xT))

        dv = np.zeros(SHARD, dtype=np.float32)
        dv[mask] = dinv[nodes[mask]]
        dinvb_maps.append(np.ascontiguousarray(
            np.broadcast_to(dv[None, :], (128, SHARD))))
        dinvc_maps.append(np.ascontiguousarray(
            dv.reshape(G, 128).T))                       # [128, G]

    # ---- selection matrices (ones at [s, 127 + s//d]); chunk at column
    # base uses slice [:, 127-base : 255-base]
    w_ext = {}
    for d in d_set:
        w = np.zeros((128, 384), dtype=np.float32)
        s = np.arange(128)
        w[s, 127 + s // d] = 1.0
        w_ext[d] = w

    return dict(
        sched=sched, tot_slots=tot_slots, d_set=d_set, w_ext=w_ext,
        has_p=has_p, idx_maps=idx_maps, xT_maps=xT_maps,
        dinvb_maps=dinvb_maps, dinvc_maps=dinvc_maps,
        owner=owner, slot=slot, node_of_slot=node_of_slot, dinv=dinv,
        row_in_bank=row_in_bank, side=side,
    )


# ==================================================================== device
def _build_nc(prep):
    sched = prep["sched"]
    d_set = prep["d_set"]
    tot_slots = prep["tot_slots"]
    has_p = prep["has_p"]

    nc = bacc.Bacc("TRN2", target_bir_lowering=False, num_devices=NC,
                   num_swdge_queues=NQ)
    core_ids = list(range(NC))

    # ---- I/O
    xT_in = nc.declare_dram_parameter("xT", [D, SHARD], BF16, isOutput=False)
    w1_in = nc.declare_dram_parameter("W1", [D, D], BF16, isOutput=False)
    dinvc_in = nc.declare_dram_parameter("dinv_c", [128, G], F32,
                                         isOutput=False)
    w2_in = nc.declare_dram_parameter("W2", [D, D], BF16, isOutput=False)
    wl_in = nc.declare_dram_parameter("Wl", [D, 1], BF16, isOutput=False)
    dinvb_in = nc.declare_dram_parameter("dinv_b", [128, SHARD], F32,
                                         isOutput=False)
    b1c_in = nc.declare_dram_parameter("b1c", [128, 1], F32, isOutput=False)
    blr_in = nc.declare_dram_parameter("blr", [128, 1], F32, isOutput=False)
    idx_in = nc.declare_dram_parameter(
        "idx_all", [128, tot_slots // 16], I16, isOutput=False)
    wexts_in = {
        d: nc.declare_dram_parameter(f"w_ext_{d}", [128, 384], BF16,
                                     isOutput=False)
        for d in d_set
    }
    ident_in = nc.declare_dram_parameter("ident", [128, 128], BF16,
                                         isOutput=False)
    out_ext = nc.declare_dram_parameter("out", [128, G], F32, isOutput=True)

    # ---- internal DRAM
    hs_c = {}
    ag_c = {}
    for li in (1, 2):
        hs_c[(li, 0)] = nc.dram_tensor(f"hs{li}_c0", [SA, D], BF16)
        hs_c[(li, 1)] = nc.dram_tensor(f"hs{li}_c1", [SB, D], BF16)
        ag_c[(li, 0)] = nc.dram_tensor(f"hs{li}_ag0", [BANK_ROWS[0], D], BF16,
                                       addr_space="Shared")
        ag_c[(li, 1)] = nc.dram_tensor(f"hs{li}_ag1", [BANK_ROWS[1], D], BF16,
                                       addr_space="Shared")

    from contextlib import ExitStack
    with tile.TileContext(nc) as tc, ExitStack() as es:
        cpool = es.enter_context(tc.tile_pool(name="const", bufs=1))
        gpool = es.enter_context(tc.tile_pool(name="gather", bufs=GBUFS))
        spool = es.enter_context(tc.tile_pool(name="stage", bufs=6))
        ppool = es.enter_context(tc.tile_pool(name="psum", bufs=4,
                                              space="PSUM"))
        ppool2 = es.enter_context(tc.tile_pool(name="psum2", bufs=2,
                                               space="PSUM"))

        # ---------------- persistent SBUF.  Sync ring: phase-B-critical
        # loads (xT, w1, dinvc).  Scalar ring: idx/wext and the rest (first
        # gather windows need them only once the first AG lands).
        xT_t = cpool.tile([D, SHARD], BF16, tag="xT")
        nc.sync.dma_start(out=xT_t[:], in_=xT_in[:])
        w1_t = cpool.tile([D, D], BF16, tag="w1")
        nc.sync.dma_start(out=w1_t[:], in_=w1_in[:])
        dinvc_t = cpool.tile([128, G], F32, tag="dinvc")
        nc.sync.dma_start(out=dinvc_t[:], in_=dinvc_in[:])

        idx_t = cpool.tile([128, tot_slots // 16], I16, tag="idx")
        nc.scalar.dma_start(out=idx_t[:], in_=idx_in[:])
        wext_t = {}
        for d in d_set:
            t = cpool.tile([128, 384], BF16, tag=f"wext{d}")
            nc.scalar.dma_start(out=t[:], in_=wexts_in[d][:])
            wext_t[d] = t
        ident_t = cpool.tile([128, 128], BF16, tag="ident")
        nc.scalar.dma_start(out=ident_t[:], in_=ident_in[:])
        w2_t = cpool.tile([D, D], BF16, tag="w2")
        nc.scalar.dma_start(out=w2_t[:], in_=w2_in[:])
        wl_t = cpool.tile([D, 1], BF16, tag="wl")
        nc.scalar.dma_start(out=wl_t[:], in_=wl_in[:])
        dinvb_t = cpool.tile([128, SHARD], F32, tag="dinvb")
        nc.scalar.dma_start(out=dinvb_t[:], in_=dinvb_in[:])
        b1c_t = cpool.tile([128, 1], F32, tag="b1c")
        nc.scalar.dma_start(out=b1c_t[:], in_=b1c_in[:])
        blr_t = cpool.tile([128, 1], F32, tag="blr")
        nc.scalar.dma_start(out=blr_t[:], in_=blr_in[:])

        hs1_sb = cpool.tile([128, G * D], BF16, tag="hs1sb")
        hs2_sb = cpool.tile([128, G * D], BF16, tag="hs2sb")
        h1sT = cpool.tile([128, G * 128], BF16, tag="h1sT")
        park = cpool.tile([128, G * 128], F32, tag="park")
        out_sb = cpool.tile([128, G], F32, tag="outsb")

        wctr = [0]          # global gather-window counter
        gsems = [nc.alloc_semaphore(f"gsem{i}") for i in range(8)]
        gsem_uses = [0] * 8

        # ---------------- phase B: hs1 = dinv * (x @ W1)  [slots, D] bf16
        def phase_b_group(g):
            ps = ppool2.tile([128, D], F32, space="PSUM", tag="aux")
            nc.tensor.matmul(ps[:], lhsT=xT_t[:, g * 128:(g + 1) * 128],
                             rhs=w1_t[:], start=True, stop=True)
            dstv = hs1_sb[:, g * D:(g + 1) * D]
            nc.scalar.activation(dstv, ps[:], A.Copy, bias=0.0,
                                 scale=dinvc_t[:, g:g + 1])
            # bank writes are batched per side (one DMA each) right before
            # the AG trigger — 2 issue slots on the sync ring instead of 49.
            if g == GA - 1:
                nc.sync.dma_start(
                    out=hs_c[(1, 0)][:].rearrange("(k p) f -> p k f", p=128),
                    in_=hs1_sb[:, :GA * D].rearrange("p (k f) -> p k f", f=D))
            if g == G - 1:
                nc.sync.dma_start(
                    out=hs_c[(1, 1)][:].rearrange("(k p) f -> p k f", p=128),
                    in_=hs1_sb[:, GA * D:].rearrange("p (k f) -> p k f", f=D))

        def fire_ags(li, g):
            if g == GA - 1:
                nc.gpsimd.collective_compute(
                    "AllGather", mybir.AluOpType.bypass,
                    replica_groups=[core_ids],
                    ins=[hs_c[(li, 0)][:]], outs=[ag_c[(li, 0)][:]])
            if g == G - 1:
                nc.gpsimd.collective_compute(
                    "AllGather", mybir.AluOpType.bypass,
                    replica_groups=[core_ids],
                    ins=[hs_c[(li, 1)][:]], outs=[ag_c[(li, 1)][:]])

        for g in range(G):
            phase_b_group(g)
            fire_ags(1, g)

        # ---------------- message passing
        # Per chunk: mm into the group of abs_base, plus a second mm when the
        # chunk's real coverage straddles into the next group (same gathered
        # data, shifted wext slice).
        # Each job: (ci, g, d, cb, c0, c1) — the matmul writes psT[:, c0:c1]
        # of group g using wext cols [(127-cb)+c0 : (127-cb)+c1].  Chunks
        # cover disjoint dst columns within a pass, so non-first matmuls can
        # be narrow (start=False writes land on has_written=0 cols ->
        # overwrite; on has_written=1 cols from the full-width first write ->
        # accumulate).  The first matmul per (pass-1) group stays full-width
        # with start=True so stale PSUM state is cleared everywhere.
        mm_jobs = {0: [], 1: []}      # p -> [(ci, g, d, cb, c0, c1)]
        n_mm_of = {}
        chunks_of = {0: [], 1: []}
        for (p, d, base, take, off) in sched:
            chunks_of[p].append((d, base, take, off))
        for p in (0, 1):
            for ci, (d, base, take, off) in enumerate(chunks_of[p]):
                g0, cb = base // 128, base % 128
                jobs = [(ci, g0, d, cb, cb, min(128, cb + take))]
                g1 = (base + take - 1) // 128
                if g1 != g0:
                    jobs.append((ci, g1, d, cb - 128, 0, cb + take - 128))
                mm_jobs[p].extend(jobs)
                for jb in jobs:
                    g = jb[1]
                    n_mm_of[(p, g)] = n_mm_of.get((p, g), 0) + 1

        def message_pass(li, self_sb, group_done):
            """li: layer (1|2); self_sb: SBUF hs tile for self loops;
            group_done(g, accT_ap or None): epilogue callback in pass 1."""
            for p in (0, 1):
                cur = {}        # g -> [psum_tile, n_remaining_mms]
                bank = ag_c[(li, p)]
                chunks = chunks_of[p]
                jobs = mm_jobs[p]
                ji = 0
                wlist = [(w0, min(GW, len(chunks) - w0))
                         for w0 in range(0, len(chunks), GW)]

                for (w0, ncnk) in wlist:
                    wch = chunks[w0:w0 + ncnk]
                    w = wctr[0]
                    wctr[0] += 1
                    q = w % NQ
                    gt = gpool.tile([128, GW * D], BF16, tag="gmsg")
                    o = wch[0][3]
                    n_idx = ncnk * 128
                    nc.gpsimd.dma_gather(
                        gt[:, :ncnk * D].rearrange("p (c f) -> p c f",
                                                   f=D),
                        bank[:],
                        idx_t[:, o // 16:(o + n_idx) // 16],
                        n_idx, n_idx, D,
                        single_packet=SP, queue_num=q)

                    while ji < len(jobs) and jobs[ji][0] < w0 + ncnk:
                        ci, g, d, cb, c0, c1 = jobs[ji]
                        ji += 1
                        if g not in cur:
                            psT = ppool.tile([128, 128], F32, space="PSUM",
                                             tag="segps",
                                             name=f"segps{li}_{p}_{g}")
                            if p == 0:
                                # self loop: psT += hs_own^T
                                nc.tensor.matmul(
                                    psT[:], lhsT=self_sb[:, g * D:(g + 1) * D],
                                    rhs=ident_t[:], start=True, stop=False)
                            cur[g] = [psT, n_mm_of[(p, g)], p != 0]
                        ent = cur[g]
                        psT = ent[0]
                        last = ent[1] == 1
                        first = ent[2]
                        lhsT = gt[:, (ci - w0) * D:(ci - w0 + 1) * D]
                        if first:
                            # full width: start=True must clear every col's
                            # stale PSUM state (zeros written where rhs has
                            # no 1s)
                            nc.tensor.matmul(
                                psT[:], lhsT=lhsT,
                                rhs=wext_t[d][:, 127 - cb:255 - cb],
                                start=True, stop=last)
                        else:
                            nc.tensor.matmul(
                                psT[:, c0:c1], lhsT=lhsT,
                                rhs=wext_t[d][:, 127 - cb + c0:127 - cb + c1],
                                start=False, stop=last)
                        ent[1] -= 1
                        ent[2] = False
                        if last:
                            del cur[g]
                            if p == 0:
                                nc.scalar.activation(
                                    park[:, g * 128:(g + 1) * 128], psT[:],
                                    A.Copy)
                            else:
                                group_done(g, psT)

                if p == 0:
                    # groups with no pass-0 coverage: park self-only psum
                    for g in range(G):
                        if not has_p[0][g]:
                            psT = ppool.tile([128, 128], F32, space="PSUM",
                                             tag="segps",
                                             name=f"segps{li}_0x_{g}")
                            nc.tensor.matmul(
                                psT[:], lhsT=self_sb[:, g * D:(g + 1) * D],
                                rhs=ident_t[:], start=True, stop=True)
                            nc.scalar.activation(
                                park[:, g * 128:(g + 1) * 128], psT[:],
                                A.Copy)
                else:
                    for g in range(G):
                        if not has_p[1][g]:
                            group_done(g, None)

        # ---------------- layer 1 epilogue + phase D, with chunked AG2
        def mk_epi1():
            def epi(g, psT):
                win = slice(g * 128, (g + 1) * 128)
                dvb = dinvb_t[:, win]
                t0 = spool.tile([128, 128], F32, tag="e0")
                if psT is not None:
                    nc.vector.tensor_add(t0[:], psT[:], park[:, win])
                    src_t = t0
                else:
                    src_t = None
                t1 = spool.tile([128, 128], F32, tag="e1")
                nc.vector.tensor_tensor(
                    out=t1[:], in0=(src_t[:] if src_t is not None
                                    else park[:, win]),
                    in1=dvb, op=mybir.AluOpType.mult)
                t2 = spool.tile([128, 128], F32, tag="e2")
                nc.scalar.activation(t2[:], t1[:], A.Relu, bias=b1c_t[:],
                                     scale=1.0)
                nc.vector.tensor_tensor(out=h1sT[:, win], in0=t2[:], in1=dvb,
                                        op=mybir.AluOpType.mult)
                # phase D: hs2 = H1s @ W2  -> [slots, D] bf16
                ps = ppool2.tile([128, D], F32, space="PSUM", tag="aux")
                nc.tensor.matmul(ps[:], lhsT=h1sT[:, win], rhs=w2_t[:],
                                 start=True, stop=True)
                dstv = hs2_sb[:, g * D:(g + 1) * D]
                nc.scalar.activation(dstv, ps[:], A.Copy)
                if g < GA:
                    nc.sync.dma_start(
                        out=hs_c[(2, 0)][g * 128:(g + 1) * 128, :], in_=dstv)
                else:
                    r = g * 128 - SA
                    nc.sync.dma_start(out=hs_c[(2, 1)][r:r + 128, :],
                                      in_=dstv)
                fire_ags(2, g)
            return epi

        message_pass(1, hs1_sb, mk_epi1())

        # ---------------- layer 2 epilogue + final head
        def mk_epi2():
            def epi(g, psT):
                win = slice(g * 128, (g + 1) * 128)
                dvb = dinvb_t[:, win]
                t0 = spool.tile([128, 128], F32, tag="e0")
                if psT is not None:
                    nc.vector.tensor_add(t0[:], psT[:], park[:, win])
                    srcv = t0[:]
                else:
                    srcv = park[:, win]
                t1 = spool.tile([128, 128], F32, tag="e1")
                nc.vector.tensor_tensor(out=t1[:], in0=srcv, in1=dvb,
                                        op=mybir.AluOpType.mult)
                # b2 is folded into blr host-side (b2 @ Wl + bl)
                h2t = spool.tile([128, 128], BF16, tag="h2t")
                nc.scalar.activation(h2t[:], t1[:], A.Copy)
                po = ppool2.tile([128, D], F32, space="PSUM", tag="aux")
                nc.tensor.matmul(po[:, 0:1], lhsT=h2t[:], rhs=wl_t[:],
                                 start=True, stop=True)
                nc.scalar.activation(out_sb[:, g:g + 1], po[:, 0:1], A.Sigmoid,
                                     bias=blr_t[:], scale=1.0)
            return epi

        message_pass(2, hs2_sb, mk_epi2())

        nc.sync.dma_start(out=out_ext[:], in_=out_sb[:])

    nc.compile()
    return nc


# ==================================================================== entry
_CACHE = {}


def kernel(x, edge_index, W1, b1, W2, b2, Wl, bl):
    import ml_dtypes as mld

    x = np.asarray(x, dtype=np.float32)
    edge_index = np.asarray(edge_index)
    W1 = np.asarray(W1, dtype=np.float32)
    W2 = np.asarray(W2, dtype=np.float32)
    Wl = np.asarray(Wl, dtype=np.float32)
    b1 = np.asarray(b1, dtype=np.float32)
    b2 = np.asarray(b2, dtype=np.float32)
    bl = np.asarray(bl, dtype=np.float32)

    prep = _host_prep(x, edge_index)
    nc = _build_nc(prep)

    b1c = b1.reshape(D, 1).astype(np.float32)
    bl_eff = float(bl.reshape(-1)[0]) + float(b2 @ Wl.reshape(-1))
    blr = np.full((128, 1), bl_eff, dtype=np.float32)

    in_maps = []
    for c in range(NC):
        m = {
            "xT": prep["xT_maps"][c].astype(mld.bfloat16),
            "W1": W1.astype(mld.bfloat16),
            "W2": W2.astype(mld.bfloat16),
            "Wl": Wl.reshape(D, 1).astype(mld.bfloat16),
            "dinv_c": prep["dinvc_maps"][c],
            "dinv_b": prep["dinvb_maps"][c],
            "b1c": b1c, "blr": blr,
            "idx_all": prep["idx_maps"][c],
            "ident": np.eye(128, dtype=mld.bfloat16),
        }
        for d, w in prep["w_ext"].items():
            m[f"w_ext_{d}"] = np.asarray(w, dtype=mld.bfloat16)
        in_maps.append(m)

    trace = bool(os.environ.get("GNN_TRACE"))
    kw = {}
    if trace:
        kw = dict(trace=True, tmpdir=os.environ.get("GNN_TRACE_DIR") or None)
    res = run_bass_kernel_spmd(nc, in_maps, list(range(NC)), **kw)
    _CACHE["last_result"] = res
    _CACHE["prep"] = prep

    out = np.empty((N_REAL, 1), dtype=np.float32)
    owner, slot = prep["owner"], prep["slot"]
    for c in range(NC):
        o = res.results[c]["out"]          # [128, G]
        mine = np.nonzero(owner == c)[0]
        s = slot[mine]
        out[mine, 0] = o[s % 128, s // 128]
    return out


if __name__ == "__main__":
    rng = np.random.default_rng(0)
    x = rng.standard_normal((N_REAL, D), dtype=np.float32)
    ei = rng.integers(0, N_REAL, size=(2, E_EDGES), dtype=np.int64)
    W1 = rng.standard_normal((D, D), dtype=np.float32) / np.sqrt(D)
    W2 = rng.standard_normal((D, D), dtype=np.float32) / np.sqrt(D)
    Wl = rng.standard_normal((D, 1), dtype=np.float32) / np.sqrt(D)
    z = np.zeros(D, dtype=np.float32)
    out = kernel(x=x, edge_index=ei, W1=W1, b1=z, W2=W2, b2=z,
                 Wl=Wl, bl=np.zeros(1, dtype=np.float32))
    print(out.shape, out[:5, 0])

